# revision 1
# baseline (speedup 1.0000x reference)
"""Contrastive loss (InfoNCE-style logsumexp of cosine-similarity matrix) on
8 Trainium2 NeuronCores.

loss = -mean_i logsumexp_j( cos(z1_i, z2_j) / 0.05 ),  z1,z2: [8192, 512] f32

Strategy: shard z1 row-wise (1024 rows/core), replicate z2. Each core, fully
pipelined at supergroup (8-row-block / 1024-column) granularity:
  1. streams z2 in 1 MiB batched DMAs; row sum-of-squares split across ACT
     (fused Square+accum) and GpSimd-mult + DVE-reduce; batched Sqrt + DVE
     reciprocal; row-scale on GpSimd,
  2. z1 is fed RAW (its 20/||z1_i|| scale is applied later as the
     per-partition Exp scale); PE-transposes both to d-major layout through
     2-bank PSUM tiles; batched PSUM->SBUF copies convert to float32r
     (rounded fp32, 11-bit mantissa -> 4x faster PE datapath),
  3. sim block-row: f32r matmuls (K=512 via 4 accumulating chunks) into
     [128, 1024] 2-bank PSUM tiles, emitted right after the z2 columns they
     need are ready, so the PE alternates transposes and matmuls with no
     phase barriers,
  4. one ACT Exp per tile, in place, scale=20/||z1_i||, fused row-sum
     (accum_out); logsumexp without max-subtraction (|sim| <= 20 ->
     exp <= 5e8, safe in fp32),
  5. reduce + Ln -> per-row lse [128, 8] -> DRAM.
Host gathers the 8 lse tiles and returns -mean.
"""
import sys

sys.path.insert(0, "/opt/trn_rl_repo")
import numpy as np
import concourse.bacc as bacc
import concourse.mybir as mybir
from concourse import tile, masks
from concourse.bass_utils import run_bass_kernel_spmd

F32 = mybir.dt.float32
F32R = mybir.dt.float32r
AF = mybir.ActivationFunctionType
ALU = mybir.AluOpType

N, D, C = 8192, 512, 8
NS = N // C            # 1024 z1 rows per core
IB = NS // 128         # 8 i-blocks per core
NB2 = N // 128         # 64 z2 row-blocks
JH = 8                 # j-supergroups of 1024 columns (2-bank PSUM tiles)
INV_TEMP = 20.0        # 1 / 0.05


def _build():
    nc = bacc.Bacc("TRN2", target_bir_lowering=False, debug=False, num_devices=C)
    z1_d = nc.dram_tensor("z1s", [NS, D], F32, kind="ExternalInput").ap()
    z2_d = nc.dram_tensor("z2", [N, D], F32, kind="ExternalInput").ap()
    lse_d = nc.dram_tensor("lse", [128, IB], F32, kind="ExternalOutput").ap()

    with tile.TileContext(nc) as tc:
        with (
            tc.tile_pool(name="const", bufs=1) as cpool,
            tc.tile_pool(name="stage", bufs=4) as stg,
            tc.tile_pool(name="hat", bufs=4) as hat,
            tc.tile_pool(name="sqs", bufs=2) as sqs,
            tc.tile_pool(name="pbig", bufs=4, space="PSUM") as pbig,
        ):
            ident = cpool.tile([128, 128], F32)
            masks.make_identity(nc, ident[:])

            z1T = cpool.tile([128, 4 * NS], F32R, name="z1T")    # [d, (k, i)]
            z2T = cpool.tile([128, 4 * N], F32R, name="z2T")     # [d, (k, j)]
            z1Tk = z1T[:].rearrange("p (k i) -> p k i", k=4)
            z2Tk = z2T[:].rearrange("p (k j) -> p k j", k=4)
            z1Tb = z1T[:].rearrange("p (k nb i) -> p nb k i", k=4, i=128)
            z2Tb = z2T[:].rearrange("p (k nb i) -> p nb k i", k=4, i=128)

            n1sq = cpool.tile([128, IB], F32, name="n1sq")
            n1s = cpool.tile([128, IB], F32, name="n1s")
            rn1 = cpool.tile([128, IB], F32, name="rn1")
            n2sq = cpool.tile([128, NB2], F32, name="n2sq")
            n2s = cpool.tile([128, NB2], F32, name="n2s")
            rn2 = cpool.tile([128, NB2], F32, name="rn2")
            esums = cpool.tile([128, IB * JH], F32, name="esums")
            stot = cpool.tile([128, IB], F32, name="stot")
            lse_s = cpool.tile([128, IB], F32, name="lse_s")

            psv = "p (nb k i) -> p nb k i"

            def sumsq(st, n, nsq_col, b):
                blk = st[:, n * D:(n + 1) * D]
                sq = sqs.tile([128, D], F32, tag="sq", name="sq_scr")
                if b % 2 == 0:
                    nc.scalar.activation(sq[:], blk, AF.Square, accum_out=nsq_col)
                else:
                    nc.gpsimd.tensor_mul(sq[:], blk, blk)
                    nc.vector.reduce_sum(nsq_col, sq[:], axis=mybir.AxisListType.X)

            def transpose2(src_aps, name):
                # 8 PE transposes (2 row-blocks x 4 d-chunks) -> one 2-bank tile
                ps = pbig.tile([128, 1024], F32, tag="big", name=name)
                for n in range(2):
                    for k in range(4):
                        nc.tensor.transpose(
                            ps[:, (n * 4 + k) * 128:(n * 4 + k + 1) * 128],
                            src_aps[n][:, k * 128:(k + 1) * 128], ident[:])
                return ps

            z1r = z1_d.rearrange("(g n p) d -> g p n d", n=4, p=128)
            z2r = z2_d.rearrange("(g n p) d -> g p n d", n=4, p=128)
            z2st = {}

            def z1_group(g):
                # raw transposes straight off the staged tile (no normalize)
                st = stg.tile([128, 4 * D], F32, tag="stage", name=f"st1_{g}")
                nc.sync.dma_start(out=st[:].rearrange("p (n d) -> p n d", n=4),
                                  in_=z1r[g])
                for h in range(2):
                    b0 = 4 * g + 2 * h
                    ps = transpose2([st[:, (2 * h) * D:(2 * h + 1) * D],
                                     st[:, (2 * h + 1) * D:(2 * h + 2) * D]],
                                    f"ps1_{g}_{h}")
                    nc.scalar.copy(z1Tb[:, b0:b0 + 2],
                                   ps[:].rearrange(psv, nb=2, k=4))
                for n in range(4):
                    sumsq(st, n, n1sq[:, 4 * g + n:4 * g + n + 1], 4 * g + n)

            def z2_load(g):
                st = stg.tile([128, 4 * D], F32, tag="stage", name=f"st2_{g}")
                nc.sync.dma_start(out=st[:].rearrange("p (n d) -> p n d", n=4),
                                  in_=z2r[g])
                z2st[g] = st
                for n in range(4):
                    sumsq(st, n, n2sq[:, 4 * g + n:4 * g + n + 1], 4 * g + n)

            def z2_finish(gs):
                s = slice(4 * gs[0], 4 * gs[-1] + 4)
                nc.scalar.activation(n2s[:, s], n2sq[:, s], AF.Sqrt)
                nc.vector.reciprocal(rn2[:, s], n2s[:, s])
                for gg in gs:
                    st = z2st.pop(gg)
                    zhs = []
                    for n in range(4):
                        b = 4 * gg + n
                        zh = hat.tile([128, D], F32, tag="hat", name="zh")
                        nc.gpsimd.tensor_scalar(
                            zh[:], st[:, n * D:(n + 1) * D],
                            rn2[:, b:b + 1], 1.0, op0=ALU.mult, op1=ALU.mult)
                        zhs.append(zh)
                    for h in range(2):
                        b0 = 4 * gg + 2 * h
                        ps = transpose2(zhs[2 * h:2 * h + 2], f"ps2_{gg}_{h}")
                        nc.vector.tensor_copy(z2Tb[:, b0:b0 + 2],
                                              ps[:].rearrange(psv, nb=2, k=4))

            def main_tile(ib, jh):
                # [128, 1024] sim tile: 2 j-groups of 512, K=512 via 4 chunks
                ps = pbig.tile([128, 1024], F32, tag="big", name=f"mm{ib}_{jh}")
                for k in range(4):
                    for jq in range(2):
                        jb = jh * 2 + jq
                        nc.tensor.matmul(
                            ps[:, jq * 512:(jq + 1) * 512],
                            lhsT=z1Tk[:, k, ib * 128:(ib + 1) * 128],
                            rhs=z2Tk[:, k, jb * 512:(jb + 1) * 512],
                            start=(k == 0), stop=(k == 3),
                            skip_group_check=True)
                nc.scalar.activation(
                    ps[:], ps[:], AF.Exp, scale=rn1[:, ib:ib + 1],
                    accum_out=esums[:, ib * JH + jh:ib * JH + jh + 1])

            # ---------- emission: z2-first startup, then supergroup stream
            z2_load(0)
            z2_load(1)
            z2_finish([0, 1])
            z1_group(0)
            z1_group(1)
            # rn1 = 20 / ||z1_i||: sqrt(nsq/400) then reciprocal
            nc.scalar.activation(n1s[:], n1sq[:], AF.Sqrt, scale=1.0 / 400.0)
            nc.vector.reciprocal(rn1[:], n1s[:])
            for ib in range(IB):
                main_tile(ib, 0)
            for g in range(2, 2 * JH):
                z2_load(g)
                if g % 2 == 1:
                    z2_finish([g - 1, g])
                    jh = g // 2
                    for ib in range(IB):
                        main_tile(ib, jh)

            # ---------- logsumexp tail
            nc.vector.reduce_sum(stot[:], esums[:].rearrange("p (a b) -> p a b", b=JH),
                                 axis=mybir.AxisListType.X)
            nc.scalar.activation(lse_s[:], stot[:], AF.Ln)
            nc.sync.dma_start(out=lse_d[:], in_=lse_s[:])

    nc.compile()
    return nc


_nc = None


def _get_nc():
    global _nc
    if _nc is None:
        _nc = _build()
    return _nc


def kernel(z1: np.ndarray, z2: np.ndarray, _trace: bool = False, **_):
    nc = _get_nc()
    z1 = np.ascontiguousarray(z1, dtype=np.float32)
    z2 = np.ascontiguousarray(z2, dtype=np.float32)
    in_maps = [
        {"z1s": z1[c * NS:(c + 1) * NS], "z2": z2} for c in range(C)
    ]
    res = run_bass_kernel_spmd(nc, in_maps, list(range(C)), trace=_trace)
    total = 0.0
    for c in range(C):
        total += res.results[c]["lse"].astype(np.float64).sum()
    out = np.float32(-(total / N))
    if _trace:
        return out, res
    return out

